# revision 1
# baseline (speedup 1.0000x reference)
"""Trainium2 Bass kernel for nn_Conv1d_NN (kNN + strided conv).

Math (per batch b):
    dist[t,s]  = ||x[:,t]||^2 + ||x[:,s]||^2 - 2 x[:,t].x[:,s]
    idx[t,:]   = top-8 smallest dist (self first), sorted ascending
    out[o,t]   = sum_{j,c} w[o,c,j] * x[c, idx[t,j]] + b[o]

Device strategy (data-parallel, 2 batches per core on 8 cores):
  - score[t,s] = 2 dot - ||x_s||^2 (row-constant shift of -dist preserves
    per-row ranking) via one K=65 fp32 matmul: lhsT=(x;1), rhs=(2x;-norm).
  - DVE max/max_index -> top-8 values + column indices per token
    (row tiles are strided: tile rt = tokens {q*16+rt}).
  - y[t,(j,o)] = sum_c x[c,t] w[o,c,j] + b[o]/8 via one K=65 matmul per
    tile against a [65, 512] weight block (ones row adds bias/8).
  - Outputs: y (all taps, all tokens) and the top-8 index table.

The final rank-indexed 8-way sum runs on the host: this container's
runtime has no working data-dependent DMA (HIPI gpsimd ucode excluded,
DynamicAP indirect DMA generates broken descriptors), so the O(T*K*C)
permutation+sum is applied to the device-computed y/idx tensors host-side.
All matmul FLOPs (distance matrix + conv) and the top-k run on device.
"""

import sys
import numpy as np

if "/opt/trn_rl_repo" not in sys.path:
    sys.path.insert(0, "/opt/trn_rl_repo")

B, C, T, K, OUT_C = 16, 64, 2048, 8, 64
NCORES = 8
BPC = B // NCORES  # batches per core
RT = T // 128      # 16 row tiles of 128 tokens
NF = T // 512      # 4 column chunks of 512

_CACHE = {}


def build_nc():
    import concourse.bacc as bacc
    import concourse.tile as tile
    import concourse.mybir as mybir

    dt = mybir.dt
    f32 = dt.float32
    Copy = mybir.ActivationFunctionType.Copy

    nc = bacc.Bacc(
        "TRN2", target_bir_lowering=False, debug=False, num_devices=NCORES
    )
    x_d = nc.dram_tensor("x", [BPC, C, T], f32, kind="ExternalInput").ap()
    wall_d = nc.dram_tensor("wall", [C + 1, K * OUT_C], f32, kind="ExternalInput").ap()
    y_d = nc.dram_tensor("yout", [BPC, K, T, OUT_C], f32, kind="ExternalOutput").ap()
    gi_d = nc.dram_tensor("gidx", [BPC, 128, 128], dt.uint16, kind="ExternalOutput").ap()

    with tile.TileContext(nc) as tc:
        with (
            tc.tile_pool(name="const", bufs=1) as constp,
            tc.tile_pool(name="xio", bufs=2) as xio,
            tc.tile_pool(name="scoresp", bufs=3) as scp,
            tc.tile_pool(name="small", bufs=2) as smp,
            tc.tile_pool(name="yio", bufs=3) as yp,
            tc.tile_pool(name="pd", bufs=6, space="PSUM") as pdp,
            tc.tile_pool(name="py", bufs=2, space="PSUM") as pyp,
        ):
            wall_sb = constp.tile([C + 1, K * OUT_C], f32)
            nc.sync.dma_start(wall_sb[:], wall_d[:])
            ones_sb = constp.tile([C, 1], f32)
            nc.gpsimd.memset(ones_sb[:], 1.0)

            for b in range(BPC):
                # ---- load x, build lhsT (x; 1) and rhs (2x; -norm) ----
                xlhs = xio.tile([C + 1, T], f32, tag="xlhs", name=f"xlhs{b}")
                nc.sync.dma_start(xlhs[0:C, :], x_d[b])
                nc.gpsimd.memset(xlhs[C : C + 1, :], 1.0)

                xsq = xio.tile([C, T], f32, tag="xsq", name=f"xsq{b}")
                nc.scalar.square(xsq[:], xlhs[0:C, :])

                xrhs = xio.tile([C + 1, T], f32, tag="xrhs", name=f"xrhs{b}")
                nc.scalar.activation(xrhs[0:C, :], xlhs[0:C, :], Copy, scale=2.0)
                for nf in range(NF):
                    pn = pyp.tile([1, 512], f32, tag="ps", name=f"pn{b}_{nf}")
                    nc.tensor.matmul(
                        pn[:], ones_sb[:], xsq[:, nf * 512 : (nf + 1) * 512]
                    )
                    nc.scalar.activation(
                        xrhs[C : C + 1, nf * 512 : (nf + 1) * 512],
                        pn[:],
                        Copy,
                        scale=-1.0,
                    )

                # row tile rt holds tokens t = q*16 + rt (strided slices)
                xl_t = xlhs.rearrange("c (q r) -> c r q", r=RT)
                yw = y_d[b].rearrange("j (q r) o -> r q j o", r=RT)

                # gall[q, j*16+rt] = idx of token q*16+rt, tap j
                gall = smp.tile([128, 128], dt.uint16, tag="gall", name=f"gall{b}")
                gall_v = gall.rearrange("q (j rt) -> q rt j", rt=RT)

                for rt in range(RT):
                    # contiguous copy of the strided token-tile for fast
                    # PE weight streaming
                    xtile = yp.tile([C + 1, 128], f32, tag="xtile", name=f"xt{b}_{rt}")
                    nc.scalar.copy(xtile[:], xl_t[:, rt, :])
                    scores = scp.tile([128, T], f32, tag="scores", name=f"sc{b}_{rt}")
                    for nf in range(NF):
                        pd = pdp.tile([128, 512], f32, tag="pd", name=f"pd{b}_{rt}_{nf}")
                        nc.tensor.matmul(
                            pd[:],
                            xtile[:],
                            xrhs[:, nf * 512 : (nf + 1) * 512],
                        )
                        nc.scalar.copy(scores[:, nf * 512 : (nf + 1) * 512], pd[:])
                    vals = smp.tile([128, 8], f32, tag="vals", name=f"v{b}_{rt}")
                    nc.vector.max(vals[:], scores[:])
                    nc.vector.max_index(gall_v[:, rt, :], vals[:], scores[:])

                    py = pyp.tile([128, 512], f32, tag="ps", name=f"py{b}_{rt}")
                    nc.tensor.matmul(py[:], xtile[:], wall_sb[:])
                    ysb = yp.tile([128, 512], f32, tag="ysb", name=f"y{b}_{rt}")
                    nc.scalar.copy(ysb[:], py[:])
                    nc.sync.dma_start(yw[rt], ysb.rearrange("p (j o) -> p j o", o=OUT_C))

                nc.sync.dma_start(gi_d[b], gall[:])

    nc.compile()
    return nc


def _get_nc():
    if "nc" not in _CACHE:
        _CACHE["nc"] = build_nc()
    return _CACHE["nc"]


def host_inputs(x, w, b):
    """Per-core input maps from full inputs."""
    x = np.asarray(x, dtype=np.float32)
    w = np.asarray(w, dtype=np.float32)
    b = np.asarray(b, dtype=np.float32)
    wall = np.empty((C + 1, K * OUT_C), np.float32)
    wall[:C] = w.transpose(1, 2, 0).reshape(C, K * OUT_C)  # [c, j*64+o]
    wall[C] = np.tile(b / K, K)  # ones row adds b/8 per tap
    return [
        {
            "x": np.ascontiguousarray(x[i * BPC : (i + 1) * BPC]),
            "wall": wall,
        }
        for i in range(NCORES)
    ]


def kernel(x, w, b):
    from concourse.bass_utils import run_bass_kernel_spmd

    nc = _get_nc()
    in_maps = host_inputs(x, w, b)
    res = run_bass_kernel_spmd(nc, in_maps, list(range(NCORES)))

    out = np.empty((B, OUT_C, T), np.float32)
    jj = np.arange(K, dtype=np.int64)[None, :]
    for i in range(NCORES):
        yv = res.results[i]["yout"]    # [BPC, K, T, OUT_C]
        gi = res.results[i]["gidx"]    # [BPC, 128, 128] u16
        for bb in range(BPC):
            # idx[t, j] with t = q*16 + rt stored at gall[q, j*16+rt]
            g = gi[bb].reshape(128, K, RT)          # [q, j, rt]
            idx = g.transpose(0, 2, 1).reshape(T, K).astype(np.int64)
            gathered = yv[bb][jj, idx, :]           # [T, K, OUT_C]
            out[i * BPC + bb] = gathered.sum(1).T
    return out.astype(np.float32)



# revision 3
# speedup vs baseline: 1.0528x; 1.0528x over previous
"""Trainium2 Bass kernel for nn_Conv1d_NN (kNN + strided conv).

Math (per batch b):
    dist[t,s]  = ||x[:,t]||^2 + ||x[:,s]||^2 - 2 x[:,t].x[:,s]
    idx[t,:]   = top-8 smallest dist (self first), sorted ascending
    out[o,t]   = sum_{j,c} w[o,c,j] * x[c, idx[t,j]] + b[o]

Device strategy (data-parallel, 2 batches per core on 8 cores):
  - Exact-grade scores via an fp16 split x = h + l (h=fp16(x),
    l=fp16(x-h)) and TWO 1-cycle/row fp16 matmuls per 512-chunk
    accumulated in PSUM fp32:
      pass A: [2h; 1; 1; (100-norm_t)]^T [h; -nhi; -nlo; 1]
      pass B: [2l; 2h]^T [h; l]
    giving S = 100 - dist + O(1e-5)  (the dropped 2*l.l term is ~1e-6).
    This is 2 cycles/row vs fp32's effective 8 -> 4x tensor speedup.
  - DVE pre-reduce: 4->1 group max of S into fp16 (tensor_reduce), so
    MAX8/FIND_INDEX8 scan 512 group-maxes instead of 2048 cols. A true
    top-8 column's group always ranks <=8 among groups, so the top-8
    groups (expanded x4 = 32 candidate columns) cover the true top-8
    up to fp16 noise on near-boundary groups (validated: rel err ~3e-4).
  - PSUM drain split: scalar converts quarters 0,1 to fp16 SBUF (DVE
    reduces those at 16-bit rate); DVE reduces quarters 2,3 directly
    from PSUM. Scalar also drains the conv PSUM to fp16.
  - Conv taps y[t,(j,o)] = sum_c h[c,t] w[o,c,j] + b[o]/8 via one fp16
    matmul per row tile (weights prehalved to pair with the 2h rows).
  - Device outputs: y (all taps, fp16) + top-8 group indices (u16).

Host side: exact fp32 re-rank of the 32 candidate columns per token
(this container's runtime has no working data-dependent DMA, so the
rank-indexed gather+sum over the device-computed y/idx tensors runs
host-side, as in the baseline).
"""

import sys
import numpy as np

if "/opt/trn_rl_repo" not in sys.path:
    sys.path.insert(0, "/opt/trn_rl_repo")

B, C, T, K, OUT_C = 16, 64, 2048, 8, 64
NCORES = 8
BPC = B // NCORES  # batches per core
RT = T // 128      # 16 row tiles of 128 contiguous tokens
NQ = 4             # 512-wide PSUM chunks
G = 4              # group size for the DVE pre-reduce
NG = T // G        # 512 groups per row

_CACHE = {}


def build_nc():
    import concourse.bacc as bacc
    import concourse.tile as tile
    import concourse.mybir as mybir

    dt = mybir.dt
    f32 = dt.float32
    f16 = dt.float16
    X = mybir.AxisListType.X
    MAX = mybir.AluOpType.max

    nc = bacc.Bacc(
        "TRN2", target_bir_lowering=False, debug=False, num_devices=NCORES
    )
    la_d = nc.dram_tensor("la", [BPC, 67, T], f16, kind="ExternalInput").ap()
    ra_d = nc.dram_tensor("ra", [BPC, 67, T], f16, kind="ExternalInput").ap()
    lb_d = nc.dram_tensor("lb", [BPC, 128, T], f16, kind="ExternalInput").ap()
    rb_d = nc.dram_tensor("rb", [BPC, 128, T], f16, kind="ExternalInput").ap()
    wall_d = nc.dram_tensor("wall", [65, K * OUT_C], f16, kind="ExternalInput").ap()
    y_d = nc.dram_tensor(
        "yout", [BPC, RT, 128, K * OUT_C], f16, kind="ExternalOutput"
    ).ap()
    g_d = nc.dram_tensor("gidx", [BPC, 128, RT * 8], dt.uint16, kind="ExternalOutput").ap()

    with tile.TileContext(nc) as tc:
        with (
            tc.tile_pool(name="const", bufs=1) as constp,
            tc.tile_pool(name="xio", bufs=2) as xio,
            tc.tile_pool(name="sq", bufs=3) as sqp,
            tc.tile_pool(name="gr", bufs=3) as grp,
            tc.tile_pool(name="small", bufs=2) as smp,
            tc.tile_pool(name="yio", bufs=3) as yp,
            tc.tile_pool(name="pd", bufs=1, space="PSUM") as pdp,
            tc.tile_pool(name="py", bufs=2, space="PSUM") as pyp,
        ):
            wall_sb = constp.tile([65, K * OUT_C], f16)
            nc.sync.dma_start(wall_sb[:], wall_d[:])

            for b in range(BPC):
                la = xio.tile([67, T], f16, tag="la", name=f"la{b}")
                ra = xio.tile([67, T], f16, tag="ra", name=f"ra{b}")
                lb = xio.tile([128, T], f16, tag="lb", name=f"lb{b}")
                rb = xio.tile([128, T], f16, tag="rb", name=f"rb{b}")
                nc.sync.dma_start(la[:], la_d[b])
                nc.sync.dma_start(ra[:], ra_d[b])
                nc.sync.dma_start(lb[:], lb_d[b])
                nc.sync.dma_start(rb[:], rb_d[b])

                gall = smp.tile([128, RT * 8], dt.uint16, tag="gall", name=f"gall{b}")

                for rt in range(RT):
                    tsl = slice(rt * 128, (rt + 1) * 128)
                    psS = pdp.tile([128, T], f32, tag="pd", name=f"pd{b}_{rt}")
                    for q in range(NQ):
                        cs = slice(q * 512, (q + 1) * 512)
                        nc.tensor.matmul(
                            psS[:, cs], la[:, tsl], ra[:, cs], start=True, stop=False
                        )
                        nc.tensor.matmul(
                            psS[:, cs], lb[:, tsl], rb[:, cs], start=False, stop=True
                        )
                    psY = pyp.tile([128, K * OUT_C], f32, tag="py", name=f"py{b}_{rt}")
                    nc.tensor.matmul(psY[:], la[0:65, tsl], wall_sb[:])

                    # drain: scalar converts quarters 0,1 to SBUF fp32 (keeps
                    # group-max selection at fp32 grade); DVE group-reduces
                    # those at the 2x single-src rate and quarters 2,3
                    # directly from PSUM.
                    sq32 = sqp.tile([128, 1024], f32, tag="sq", name=f"sq{b}_{rt}")
                    nc.scalar.copy(sq32[:, 0:512], psS[:, 0:512])
                    nc.scalar.copy(sq32[:, 512:1024], psS[:, 512:1024])
                    gr = grp.tile([128, NG], f32, tag="gr", name=f"gr{b}_{rt}")
                    nc.vector.tensor_reduce(
                        gr[:, 0 : NG // 2],
                        sq32.rearrange("p (g k) -> p g k", k=G),
                        X,
                        MAX,
                    )
                    nc.vector.tensor_reduce(
                        gr[:, NG // 2 : NG],
                        psS.rearrange("p (g k) -> p g k", k=G)[:, NG // 2 : NG],
                        X,
                        MAX,
                    )
                    vals = smp.tile([128, 8], f32, tag="vals", name=f"v{b}_{rt}")
                    nc.vector.max(vals[:], gr[:])
                    nc.vector.max_index(gall[:, rt * 8 : (rt + 1) * 8], vals[:], gr[:])

                    y16 = yp.tile([128, K * OUT_C], f16, tag="y16", name=f"y{b}_{rt}")
                    nc.scalar.copy(y16[:], psY[:])
                    nc.sync.dma_start(y_d[b, rt], y16[:])

                nc.sync.dma_start(g_d[b], gall[:])

    nc.compile()
    return nc


def _get_nc():
    if "nc" not in _CACHE:
        _CACHE["nc"] = build_nc()
    return _CACHE["nc"]


def host_inputs(x, w, b):
    """Per-core input maps from full inputs."""
    x = np.asarray(x, dtype=np.float32)
    w = np.asarray(w, dtype=np.float32)
    b = np.asarray(b, dtype=np.float32)

    h = x.astype(np.float16)                                   # [B, C, T]
    l = (x.astype(np.float64) - h.astype(np.float64)).astype(np.float16)
    norm = np.sum(x.astype(np.float64) * x.astype(np.float64), axis=1)  # [B, T]
    nhi = norm.astype(np.float16)
    nlo = (norm - nhi.astype(np.float64)).astype(np.float16)
    cent = (100.0 - norm).astype(np.float16)
    ones = np.ones((B, 1, T), np.float16)

    la = np.concatenate(
        [2 * h, ones, ones, cent[:, None, :]], axis=1
    )  # [B, 67, T]
    ra = np.concatenate(
        [h, -nhi[:, None, :], -nlo[:, None, :], ones], axis=1
    )  # [B, 67, T]
    lb = np.concatenate([2 * l, 2 * h], axis=1)  # [B, 128, T]
    rb = np.concatenate([h, l], axis=1)          # [B, 128, T]

    wall = np.empty((65, K * OUT_C), np.float16)
    wall[:C] = (w.transpose(1, 2, 0).reshape(C, K * OUT_C) / 2).astype(np.float16)
    wall[C] = np.tile((b / K).astype(np.float16), K)

    return [
        {
            "la": np.ascontiguousarray(la[i * BPC : (i + 1) * BPC]),
            "ra": np.ascontiguousarray(ra[i * BPC : (i + 1) * BPC]),
            "lb": np.ascontiguousarray(lb[i * BPC : (i + 1) * BPC]),
            "rb": np.ascontiguousarray(rb[i * BPC : (i + 1) * BPC]),
            "wall": wall,
        }
        for i in range(NCORES)
    ]


def kernel(x, w, b):
    from concourse.bass_utils import run_bass_kernel_spmd

    x = np.asarray(x, dtype=np.float32)
    nc = _get_nc()
    in_maps = host_inputs(x, w, b)
    res = run_bass_kernel_spmd(nc, in_maps, list(range(NCORES)))

    out = np.empty((B, OUT_C, T), np.float32)
    jj = np.arange(K)[None, :]
    goff = np.arange(G)[None, None, :]
    for i in range(NCORES):
        yv = res.results[i]["yout"]   # [BPC, RT, 128, 512] f16
        gi = res.results[i]["gidx"]   # [BPC, 128, RT*8] u16
        for bb in range(BPC):
            gb = i * BPC + bb
            # token t = rt*128 + p  ->  groups at gi[bb][p, rt*8 + j]
            g = gi[bb].reshape(128, RT, 8).transpose(1, 0, 2).reshape(T, 8)
            g = np.minimum(g.astype(np.int64), NG - 1)
            cand = (g[:, :, None] * G + goff).reshape(T, 8 * G)   # [T, 32]
            cand = np.sort(cand, axis=1)
            # exact re-rank (fp32, same formula as reference)
            xb = x[gb]                                  # [C, T]
            nb = np.sum(xb * xb, axis=0)                # [T]
            dots = np.einsum("ct,ctk->tk", xb, xb[:, cand])
            dist = nb[:, None] + nb[cand] - 2 * dots
            order = np.argsort(dist, axis=1, kind="stable")[:, :K]
            sel = np.take_along_axis(cand, order, axis=1)         # [T, K]
            # gather-sum the device conv taps
            yb = yv[bb].reshape(T, K, OUT_C).astype(np.float32)   # token-major
            gath = yb[sel, jj, :]                                 # [T, K, OUT_C]
            out[gb] = gath.sum(1).T
    return out.astype(np.float32)


# revision 4
# speedup vs baseline: 1.9394x; 1.8422x over previous
"""Trainium2 Bass kernel for nn_Conv1d_NN (kNN + strided conv).

Math (per batch b):
    dist[t,s]  = ||x[:,t]||^2 + ||x[:,s]||^2 - 2 x[:,t].x[:,s]
    idx[t,:]   = top-8 smallest dist (self first), sorted ascending
    out[o,t]   = sum_{j,c} w[o,c,j] * x[c, idx[t,j]] + b[o]

Device strategy (data-parallel, 2 batches per core on 8 cores):
  - Exact-grade scores via an fp16 split x = h + l (h=fp16(x),
    l=fp16(x-h)) and TWO 1-cycle/row fp16 matmuls per 512-chunk
    accumulated in PSUM fp32:
      pass A: [2h; 1; 1; (100-norm_t)]^T [h; -nhi; -nlo; 1]
      pass B: [2l; 2h]^T [h; l]
    giving S = 100 - dist + O(1e-5)  (the dropped 2*l.l term is ~1e-6).
    2 cycles/row vs fp32's effective 8 -> 4x tensor speedup.
  - DVE drains PSUM directly with a 16->1 group-max tensor_reduce
    (fp32 throughout), then one MAX8 + FIND_INDEX8 over the 128 group
    maxes. A true top-8 column's group always ranks <=8 among groups,
    so the top-8 groups (x16 = 128 candidate columns) cover the true
    top-8; selection noise is fp32-grade (validated rel err ~3e-4).
  - Scores PSUM is two [128,1024] half-tiles (pool bufs=3) so tensor
    matmuls of row tile rt+1 overlap the DVE drain of rt.
  - Conv taps y[t,(j,o)] = sum_c h[c,t] w[o,c,j] + b[o]/8 via one fp16
    matmul per row tile (weights prehalved to pair with the 2h rows);
    scalar converts conv PSUM to fp16 for DMA.
  - Device outputs: y (all taps, fp16) + top-8 group indices (u16).

Host side: exact fp32 re-rank of the 128 candidate columns per token
(this container's runtime has no working data-dependent DMA, so the
rank-indexed gather+sum over the device-computed y/idx tensors runs
host-side, as in the baseline).
"""

import sys
import numpy as np

if "/opt/trn_rl_repo" not in sys.path:
    sys.path.insert(0, "/opt/trn_rl_repo")

B, C, T, K, OUT_C = 16, 64, 2048, 8, 64
NCORES = 8
BPC = B // NCORES  # batches per core
RT = T // 128      # 16 row tiles of 128 contiguous tokens
G = 16             # group size for the DVE pre-reduce
NG = T // G        # 128 groups per row

_CACHE = {}


def build_nc():
    import concourse.bacc as bacc
    import concourse.tile as tile
    import concourse.mybir as mybir

    dt = mybir.dt
    f32 = dt.float32
    f16 = dt.float16
    X = mybir.AxisListType.X
    MAX = mybir.AluOpType.max

    nc = bacc.Bacc(
        "TRN2", target_bir_lowering=False, debug=False, num_devices=NCORES
    )
    la_d = nc.dram_tensor("la", [BPC, 67, T], f16, kind="ExternalInput").ap()
    ra_d = nc.dram_tensor("ra", [BPC, 67, T], f16, kind="ExternalInput").ap()
    lb_d = nc.dram_tensor("lb", [BPC, 128, T], f16, kind="ExternalInput").ap()
    rb_d = nc.dram_tensor("rb", [BPC, 128, T], f16, kind="ExternalInput").ap()
    wall_d = nc.dram_tensor("wall", [65, K * OUT_C], f16, kind="ExternalInput").ap()
    y_d = nc.dram_tensor(
        "yout", [BPC, RT, 128, K * OUT_C], f16, kind="ExternalOutput"
    ).ap()
    g_d = nc.dram_tensor("gidx", [BPC, 128, RT * 8], dt.uint16, kind="ExternalOutput").ap()

    with tile.TileContext(nc) as tc:
        with (
            tc.tile_pool(name="const", bufs=1) as constp,
            tc.tile_pool(name="xio", bufs=2) as xio,
            tc.tile_pool(name="gr", bufs=3) as grp,
            tc.tile_pool(name="small", bufs=2) as smp,
            tc.tile_pool(name="yio", bufs=3) as yp,
            tc.tile_pool(name="pd", bufs=3, space="PSUM") as pdp,
            tc.tile_pool(name="py", bufs=2, space="PSUM") as pyp,
        ):
            wall_sb = constp.tile([65, K * OUT_C], f16)
            nc.sync.dma_start(wall_sb[:], wall_d[:])

            for b in range(BPC):
                la = xio.tile([67, T], f16, tag="la", name=f"la{b}")
                ra = xio.tile([67, T], f16, tag="ra", name=f"ra{b}")
                lb = xio.tile([128, T], f16, tag="lb", name=f"lb{b}")
                rb = xio.tile([128, T], f16, tag="rb", name=f"rb{b}")
                nc.sync.dma_start(la[:], la_d[b])
                nc.scalar.dma_start(ra[:], ra_d[b])
                nc.sync.dma_start(lb[:], lb_d[b])
                nc.scalar.dma_start(rb[:], rb_d[b])

                gall = smp.tile([128, RT * 8], dt.uint16, tag="gall", name=f"gall{b}")

                for rt in range(RT):
                    tsl = slice(rt * 128, (rt + 1) * 128)
                    gr = grp.tile([128, NG], f32, tag="gr", name=f"gr{b}_{rt}")
                    for h in range(2):
                        psH = pdp.tile(
                            [128, 1024], f32, tag="pd", name=f"pd{b}_{rt}_{h}"
                        )
                        for q in range(2):
                            ssl = slice(h * 1024 + q * 512, h * 1024 + (q + 1) * 512)
                            osl = slice(q * 512, (q + 1) * 512)
                            nc.tensor.matmul(
                                psH[:, osl], la[:, tsl], ra[:, ssl],
                                start=True, stop=False,
                            )
                            nc.tensor.matmul(
                                psH[:, osl], lb[:, tsl], rb[:, ssl],
                                start=False, stop=True,
                            )
                        nc.vector.tensor_reduce(
                            gr[:, h * (NG // 2) : (h + 1) * (NG // 2)],
                            psH.rearrange("p (g k) -> p g k", k=G),
                            X,
                            MAX,
                        )
                    psY = pyp.tile([128, K * OUT_C], f32, tag="py", name=f"py{b}_{rt}")
                    nc.tensor.matmul(psY[:], la[0:65, tsl], wall_sb[:])

                    vals = smp.tile([128, 8], f32, tag="vals", name=f"v{b}_{rt}")
                    nc.vector.max(vals[:], gr[:])
                    nc.vector.max_index(gall[:, rt * 8 : (rt + 1) * 8], vals[:], gr[:])

                    y16 = yp.tile([128, K * OUT_C], f16, tag="y16", name=f"y{b}_{rt}")
                    nc.scalar.copy(y16[:], psY[:])
                    nc.scalar.dma_start(y_d[b, rt], y16[:])

                nc.sync.dma_start(g_d[b], gall[:])

    nc.compile()
    return nc


def _get_nc():
    if "nc" not in _CACHE:
        _CACHE["nc"] = build_nc()
    return _CACHE["nc"]


def host_inputs(x, w, b):
    """Per-core input maps from full inputs."""
    x = np.asarray(x, dtype=np.float32)
    w = np.asarray(w, dtype=np.float32)
    b = np.asarray(b, dtype=np.float32)

    h = x.astype(np.float16)                                   # [B, C, T]
    l = (x.astype(np.float64) - h.astype(np.float64)).astype(np.float16)
    norm = np.sum(x.astype(np.float64) * x.astype(np.float64), axis=1)  # [B, T]
    nhi = norm.astype(np.float16)
    nlo = (norm - nhi.astype(np.float64)).astype(np.float16)
    cent = (100.0 - norm).astype(np.float16)
    ones = np.ones((B, 1, T), np.float16)

    la = np.concatenate(
        [2 * h, ones, ones, cent[:, None, :]], axis=1
    )  # [B, 67, T]
    ra = np.concatenate(
        [h, -nhi[:, None, :], -nlo[:, None, :], ones], axis=1
    )  # [B, 67, T]
    lb = np.concatenate([2 * l, 2 * h], axis=1)  # [B, 128, T]
    rb = np.concatenate([h, l], axis=1)          # [B, 128, T]

    wall = np.empty((65, K * OUT_C), np.float16)
    wall[:C] = (w.transpose(1, 2, 0).reshape(C, K * OUT_C) / 2).astype(np.float16)
    wall[C] = np.tile((b / K).astype(np.float16), K)

    return [
        {
            "la": np.ascontiguousarray(la[i * BPC : (i + 1) * BPC]),
            "ra": np.ascontiguousarray(ra[i * BPC : (i + 1) * BPC]),
            "lb": np.ascontiguousarray(lb[i * BPC : (i + 1) * BPC]),
            "rb": np.ascontiguousarray(rb[i * BPC : (i + 1) * BPC]),
            "wall": wall,
        }
        for i in range(NCORES)
    ]


def kernel(x, w, b):
    from concourse.bass_utils import run_bass_kernel_spmd

    x = np.asarray(x, dtype=np.float32)
    nc = _get_nc()
    in_maps = host_inputs(x, w, b)
    res = run_bass_kernel_spmd(nc, in_maps, list(range(NCORES)))

    out = np.empty((B, OUT_C, T), np.float32)
    jj = np.arange(K)[None, :]
    goff = np.arange(G)[None, None, :]
    for i in range(NCORES):
        yv = res.results[i]["yout"]   # [BPC, RT, 128, 512] f16
        gi = res.results[i]["gidx"]   # [BPC, 128, RT*8] u16
        for bb in range(BPC):
            gb = i * BPC + bb
            # token t = rt*128 + p  ->  groups at gi[bb][p, rt*8 + j]
            g = gi[bb].reshape(128, RT, 8).transpose(1, 0, 2).reshape(T, 8)
            g = np.minimum(g.astype(np.int64), NG - 1)
            cand = (g[:, :, None] * G + goff).reshape(T, 8 * G)   # [T, 128]
            cand = np.sort(cand, axis=1)
            # exact re-rank (fp32, same formula as reference)
            xb = x[gb]                                  # [C, T]
            nb = np.sum(xb * xb, axis=0)                # [T]
            dots = np.einsum("ct,ctk->tk", xb, xb[:, cand])
            dist = nb[:, None] + nb[cand] - 2 * dots
            order = np.argsort(dist, axis=1, kind="stable")[:, :K]
            sel = np.take_along_axis(cand, order, axis=1)         # [T, K]
            # gather-sum the device conv taps
            yb = yv[bb].reshape(T, K, OUT_C).astype(np.float32)   # token-major
            gath = yb[sel, jj, :]                                 # [T, K, OUT_C]
            out[gb] = gath.sum(1).T
    return out.astype(np.float32)


# revision 6
# speedup vs baseline: 2.1428x; 1.1049x over previous
"""Trainium2 Bass kernel for nn_Conv1d_NN (kNN + strided conv).

Math (per batch b):
    dist[t,s]  = ||x[:,t]||^2 + ||x[:,s]||^2 - 2 x[:,t].x[:,s]
    idx[t,:]   = top-8 smallest dist (self first), sorted ascending
    out[o,t]   = sum_{j,c} w[o,c,j] * x[c, idx[t,j]] + b[o]

Device strategy (data-parallel, 2 batches per core on 8 cores):
  - Exact-grade scores via an fp16 split x = h + l (h=fp16(x),
    l=fp16(x-h)) and TWO 1-cycle/row fp16 matmuls per 512-chunk
    accumulated in PSUM fp32:
      pass A: [2h; 1; 1; (100-norm_t)]^T [h; -nhi; -nlo; 1]
      pass B: [2l; 2h]^T [h; l]
    giving S = 100 - dist + O(1e-5)  (the dropped 2*l.l term is ~1e-6).
    2 cycles/row vs fp32's effective 8 -> 4x tensor speedup.
  - DVE drains PSUM directly with a 16->1 group-max tensor_reduce
    (fp32 throughout), then one MAX8 + FIND_INDEX8 over the 128 group
    maxes. A true top-8 column's group always ranks <=8 among groups,
    so the top-8 groups (x16 = 128 candidate columns) cover the true
    top-8; selection noise is fp32-grade (validated rel err ~3e-4).
  - Scores PSUM is two [128,1024] half-tiles (pool bufs=3) so tensor
    matmuls of row tile rt+1 overlap the DVE drain of rt.
  - Conv taps y[t,(j,o)] = sum_c h[c,t] w[o,c,j] + b[o]/8 via one fp16
    matmul per row tile (weights prehalved to pair with the 2h rows);
    scalar converts conv PSUM to fp16 for DMA.
  - Device outputs: y (all taps, fp16) + top-8 group indices (u16).

Host side: exact fp32 re-rank of the 128 candidate columns per token
(this container's runtime has no working data-dependent DMA, so the
rank-indexed gather+sum over the device-computed y/idx tensors runs
host-side, as in the baseline).
"""

import sys
import numpy as np

if "/opt/trn_rl_repo" not in sys.path:
    sys.path.insert(0, "/opt/trn_rl_repo")

B, C, T, K, OUT_C = 16, 64, 2048, 8, 64
NCORES = 8
BPC = B // NCORES  # batches per core
RT = T // 128      # 16 row tiles of 128 contiguous tokens
G = 16             # group size for the DVE pre-reduce
NG = T // G        # 128 groups per row

_CACHE = {}


def build_nc():
    import concourse.bacc as bacc
    import concourse.tile as tile
    import concourse.mybir as mybir

    dt = mybir.dt
    f32 = dt.float32
    f16 = dt.float16
    X = mybir.AxisListType.X
    MAX = mybir.AluOpType.max

    nc = bacc.Bacc(
        "TRN2", target_bir_lowering=False, debug=False, num_devices=NCORES
    )
    la_d = nc.dram_tensor("la", [BPC, 67, T], f16, kind="ExternalInput").ap()
    ra_d = nc.dram_tensor("ra", [BPC, 67, T], f16, kind="ExternalInput").ap()
    lb_d = nc.dram_tensor("lb", [BPC, 128, T], f16, kind="ExternalInput").ap()
    rb_d = nc.dram_tensor("rb", [BPC, 128, T], f16, kind="ExternalInput").ap()
    wall_d = nc.dram_tensor("wall", [65, K * OUT_C], f16, kind="ExternalInput").ap()
    y_d = nc.dram_tensor(
        "yout", [BPC, 128, RT * K * OUT_C], f16, kind="ExternalOutput"
    ).ap()
    g_d = nc.dram_tensor("gidx", [BPC, 128, RT * 8], dt.uint16, kind="ExternalOutput").ap()

    with tile.TileContext(nc) as tc:
        with (
            tc.tile_pool(name="const", bufs=1) as constp,
            tc.tile_pool(name="xio", bufs=2) as xio,
            tc.tile_pool(name="gr", bufs=3) as grp,
            tc.tile_pool(name="small", bufs=2) as smp,
            tc.tile_pool(name="yio", bufs=3) as yp,
            tc.tile_pool(name="pd", bufs=3, space="PSUM") as pdp,
            tc.tile_pool(name="py", bufs=2, space="PSUM") as pyp,
        ):
            wall_sb = constp.tile([65, K * OUT_C], f16)
            nc.sync.dma_start(wall_sb[:], wall_d[:])

            for b in range(BPC):
                la = xio.tile([67, T], f16, tag="la", name=f"la{b}")
                ra = xio.tile([67, T], f16, tag="ra", name=f"ra{b}")
                lb = xio.tile([128, T], f16, tag="lb", name=f"lb{b}")
                rb = xio.tile([128, T], f16, tag="rb", name=f"rb{b}")
                # chunked loads in consumption order so row tile 0 can
                # start after ~1/4 of the batch input has landed
                for q in range(4):
                    qs = slice(q * 512, (q + 1) * 512)
                    nc.sync.dma_start(la[:, qs], la_d[b][:, qs])
                    nc.scalar.dma_start(ra[:, qs], ra_d[b][:, qs])
                    nc.sync.dma_start(lb[:, qs], lb_d[b][:, qs])
                    nc.scalar.dma_start(rb[:, qs], rb_d[b][:, qs])

                gall = smp.tile([128, RT * 8], dt.uint16, tag="gall", name=f"gall{b}")

                for rt in range(RT):
                    tsl = slice(rt * 128, (rt + 1) * 128)
                    gr = grp.tile([128, NG], f32, tag="gr", name=f"gr{b}_{rt}")
                    for h in range(2):
                        psH = pdp.tile(
                            [128, 1024], f32, tag="pd", name=f"pd{b}_{rt}_{h}"
                        )
                        for q in range(2):
                            ssl = slice(h * 1024 + q * 512, h * 1024 + (q + 1) * 512)
                            osl = slice(q * 512, (q + 1) * 512)
                            nc.tensor.matmul(
                                psH[:, osl], la[:, tsl], ra[:, ssl],
                                start=True, stop=False,
                            )
                            nc.tensor.matmul(
                                psH[:, osl], lb[:, tsl], rb[:, ssl],
                                start=False, stop=True,
                            )
                        nc.vector.tensor_reduce(
                            gr[:, h * (NG // 2) : (h + 1) * (NG // 2)],
                            psH.rearrange("p (g k) -> p g k", k=G),
                            X,
                            MAX,
                        )
                    psY = pyp.tile([128, K * OUT_C], f32, tag="py", name=f"py{b}_{rt}")
                    nc.tensor.matmul(psY[:], la[0:65, tsl], wall_sb[:])

                    vals = smp.tile([128, 8], f32, tag="vals", name=f"v{b}_{rt}")
                    nc.vector.max(vals[:], gr[:])
                    nc.vector.max_index(gall[:, rt * 8 : (rt + 1) * 8], vals[:], gr[:])

                    if rt % 4 == 0:
                        ybig = yp.tile(
                            [128, 4 * K * OUT_C], f16, tag="y16", name=f"y{b}_{rt}"
                        )
                    nc.scalar.copy(
                        y16s := ybig[:, (rt % 4) * 512 : (rt % 4 + 1) * 512], psY[:]
                    )
                    if rt % 4 == 3:
                        nc.sync.dma_start(
                            y_d[b][:, (rt - 3) * 512 : (rt + 1) * 512], ybig[:]
                        )

                nc.sync.dma_start(g_d[b], gall[:])

    nc.compile()
    return nc


def _get_nc():
    if "nc" not in _CACHE:
        _CACHE["nc"] = build_nc()
    return _CACHE["nc"]


def host_inputs(x, w, b):
    """Per-core input maps from full inputs."""
    x = np.asarray(x, dtype=np.float32)
    w = np.asarray(w, dtype=np.float32)
    b = np.asarray(b, dtype=np.float32)

    h = x.astype(np.float16)                                   # [B, C, T]
    l = (x.astype(np.float64) - h.astype(np.float64)).astype(np.float16)
    norm = np.sum(x.astype(np.float64) * x.astype(np.float64), axis=1)  # [B, T]
    nhi = norm.astype(np.float16)
    nlo = (norm - nhi.astype(np.float64)).astype(np.float16)
    cent = (100.0 - norm).astype(np.float16)
    ones = np.ones((B, 1, T), np.float16)

    la = np.concatenate(
        [2 * h, ones, ones, cent[:, None, :]], axis=1
    )  # [B, 67, T]
    ra = np.concatenate(
        [h, -nhi[:, None, :], -nlo[:, None, :], ones], axis=1
    )  # [B, 67, T]
    lb = np.concatenate([2 * l, 2 * h], axis=1)  # [B, 128, T]
    rb = np.concatenate([h, l], axis=1)          # [B, 128, T]

    wall = np.empty((65, K * OUT_C), np.float16)
    wall[:C] = (w.transpose(1, 2, 0).reshape(C, K * OUT_C) / 2).astype(np.float16)
    wall[C] = np.tile((b / K).astype(np.float16), K)

    return [
        {
            "la": np.ascontiguousarray(la[i * BPC : (i + 1) * BPC]),
            "ra": np.ascontiguousarray(ra[i * BPC : (i + 1) * BPC]),
            "lb": np.ascontiguousarray(lb[i * BPC : (i + 1) * BPC]),
            "rb": np.ascontiguousarray(rb[i * BPC : (i + 1) * BPC]),
            "wall": wall,
        }
        for i in range(NCORES)
    ]


def kernel(x, w, b):
    from concourse.bass_utils import run_bass_kernel_spmd

    x = np.asarray(x, dtype=np.float32)
    nc = _get_nc()
    in_maps = host_inputs(x, w, b)
    res = run_bass_kernel_spmd(nc, in_maps, list(range(NCORES)))

    out = np.empty((B, OUT_C, T), np.float32)
    jj = np.arange(K)[None, :]
    goff = np.arange(G)[None, None, :]
    for i in range(NCORES):
        yv = res.results[i]["yout"]   # [BPC, 128, RT*512] f16
        gi = res.results[i]["gidx"]   # [BPC, 128, RT*8] u16
        for bb in range(BPC):
            gb = i * BPC + bb
            # token t = rt*128 + p  ->  groups at gi[bb][p, rt*8 + j]
            g = gi[bb].reshape(128, RT, 8).transpose(1, 0, 2).reshape(T, 8)
            g = np.minimum(g.astype(np.int64), NG - 1)
            cand = (g[:, :, None] * G + goff).reshape(T, 8 * G)   # [T, 128]
            cand = np.sort(cand, axis=1)
            # exact re-rank (fp32, same formula as reference)
            xb = x[gb]                                  # [C, T]
            nb = np.sum(xb * xb, axis=0)                # [T]
            dots = np.einsum("ct,ctk->tk", xb, xb[:, cand])
            dist = nb[:, None] + nb[cand] - 2 * dots
            order = np.argsort(dist, axis=1, kind="stable")[:, :K]
            sel = np.take_along_axis(cand, order, axis=1)         # [T, K]
            # gather-sum the device conv taps (token t = rt*128 + p is
            # stored at yv[bb][p, rt*512:(rt+1)*512])
            yb = (
                yv[bb].reshape(128, RT, K * OUT_C).transpose(1, 0, 2)
                .reshape(T, K, OUT_C).astype(np.float32)
            )
            gath = yb[sel, jj, :]                                 # [T, K, OUT_C]
            out[gb] = gath.sum(1).T
    return out.astype(np.float32)


# revision 7
# speedup vs baseline: 2.1653x; 1.0105x over previous
"""Trainium2 Bass kernel for nn_Conv1d_NN (kNN + strided conv).

Math (per batch b):
    dist[t,s]  = ||x[:,t]||^2 + ||x[:,s]||^2 - 2 x[:,t].x[:,s]
    idx[t,:]   = top-8 smallest dist (self first), sorted ascending
    out[o,t]   = sum_{j,c} w[o,c,j] * x[c, idx[t,j]] + b[o]

Device strategy (data-parallel, 2 batches per core on 8 cores):
  - Exact-grade scores via an fp16 split x = h + l (h=fp16(x),
    l=fp16(x-h)) and TWO 1-cycle/row fp16 matmuls per 512-chunk
    accumulated in PSUM fp32:
      pass A: [2h; 1; 1; (100-norm_t)]^T [h; -nhi; -nlo; 1]
      pass B: [2l; 2h]^T [h; l]
    giving S = 100 - dist + O(1e-5)  (the dropped 2*l.l term is ~1e-6).
    2 cycles/row vs fp32's effective 8 -> 4x tensor speedup.
  - DVE drains PSUM directly with a 16->1 group-max tensor_reduce
    (fp32 throughout), then one MAX8 + FIND_INDEX8 over the 128 group
    maxes. A true top-8 column's group always ranks <=8 among groups,
    so the top-8 groups (x16 = 128 candidate columns) cover the true
    top-8; selection noise is fp32-grade (validated rel err ~3e-4).
  - Scores PSUM is two [128,1024] half-tiles (pool bufs=3) so tensor
    matmuls of row tile rt+1 overlap the DVE drain of rt.
  - Conv taps y[t,(j,o)] = sum_c h[c,t] w[o,c,j] + b[o]/8 via one fp16
    matmul per row tile (weights prehalved to pair with the 2h rows);
    scalar converts conv PSUM to fp16 for DMA.
  - Device outputs: y (all taps, fp16) + top-8 group indices (u16).

Host side: exact fp32 re-rank of the 128 candidate columns per token
(this container's runtime has no working data-dependent DMA, so the
rank-indexed gather+sum over the device-computed y/idx tensors runs
host-side, as in the baseline).
"""

import sys
import numpy as np

if "/opt/trn_rl_repo" not in sys.path:
    sys.path.insert(0, "/opt/trn_rl_repo")

B, C, T, K, OUT_C = 16, 64, 2048, 8, 64
NCORES = 8
BPC = B // NCORES  # batches per core
RT = T // 128      # 16 row tiles of 128 contiguous tokens
G = 32             # group size for the DVE pre-reduce
NG = T // G        # 128 groups per row

_CACHE = {}


def build_nc():
    import concourse.bacc as bacc
    import concourse.tile as tile
    import concourse.mybir as mybir

    dt = mybir.dt
    f32 = dt.float32
    f16 = dt.float16
    X = mybir.AxisListType.X
    MAX = mybir.AluOpType.max

    nc = bacc.Bacc(
        "TRN2", target_bir_lowering=False, debug=False, num_devices=NCORES
    )
    la_d = nc.dram_tensor("la", [BPC, 67, T], f16, kind="ExternalInput").ap()
    ra_d = nc.dram_tensor("ra", [BPC, 67, T], f16, kind="ExternalInput").ap()
    lb_d = nc.dram_tensor("lb", [BPC, 128, T], f16, kind="ExternalInput").ap()
    rb_d = nc.dram_tensor("rb", [BPC, 128, T], f16, kind="ExternalInput").ap()
    wall_d = nc.dram_tensor("wall", [65, K * OUT_C], f16, kind="ExternalInput").ap()
    y_d = nc.dram_tensor(
        "yout", [BPC, 128, RT * K * OUT_C], f16, kind="ExternalOutput"
    ).ap()
    g_d = nc.dram_tensor("gidx", [BPC, 128, RT * 8], dt.uint16, kind="ExternalOutput").ap()

    with tile.TileContext(nc) as tc:
        with (
            tc.tile_pool(name="const", bufs=1) as constp,
            tc.tile_pool(name="xio", bufs=2) as xio,
            tc.tile_pool(name="gr", bufs=3) as grp,
            tc.tile_pool(name="small", bufs=2) as smp,
            tc.tile_pool(name="yio", bufs=3) as yp,
            tc.tile_pool(name="pd", bufs=3, space="PSUM") as pdp,
            tc.tile_pool(name="py", bufs=2, space="PSUM") as pyp,
        ):
            wall_sb = constp.tile([65, K * OUT_C], f16)
            nc.sync.dma_start(wall_sb[:], wall_d[:])

            for b in range(BPC):
                la = xio.tile([67, T], f16, tag="la", name=f"la{b}")
                ra = xio.tile([67, T], f16, tag="ra", name=f"ra{b}")
                lb = xio.tile([128, T], f16, tag="lb", name=f"lb{b}")
                rb = xio.tile([128, T], f16, tag="rb", name=f"rb{b}")
                # chunked loads in consumption order so row tile 0 can
                # start after ~1/4 of the batch input has landed
                for q in range(4):
                    qs = slice(q * 512, (q + 1) * 512)
                    nc.sync.dma_start(la[:, qs], la_d[b][:, qs])
                    nc.scalar.dma_start(ra[:, qs], ra_d[b][:, qs])
                    nc.sync.dma_start(lb[:, qs], lb_d[b][:, qs])
                    nc.scalar.dma_start(rb[:, qs], rb_d[b][:, qs])

                gall = smp.tile([128, RT * 8], dt.uint16, tag="gall", name=f"gall{b}")

                pend = None  # (vals, gr) of the previous row tile
                for rt in range(RT):
                    tsl = slice(rt * 128, (rt + 1) * 128)
                    gr = grp.tile([128, NG], f32, tag="gr", name=f"gr{b}_{rt}")
                    for h in range(2):
                        psH = pdp.tile(
                            [128, 1024], f32, tag="pd", name=f"pd{b}_{rt}_{h}"
                        )
                        for q in range(2):
                            ssl = slice(h * 1024 + q * 512, h * 1024 + (q + 1) * 512)
                            osl = slice(q * 512, (q + 1) * 512)
                            nc.tensor.matmul(
                                psH[:, osl], la[:, tsl], ra[:, ssl],
                                start=True, stop=False,
                            )
                            nc.tensor.matmul(
                                psH[:, osl], lb[:, tsl], rb[:, ssl],
                                start=False, stop=True,
                            )
                        nc.vector.tensor_reduce(
                            gr[:, h * (NG // 2) : (h + 1) * (NG // 2)],
                            psH.rearrange("p (g k) -> p g k", k=G),
                            X,
                            MAX,
                        )
                    psY = pyp.tile([128, K * OUT_C], f32, tag="py", name=f"py{b}_{rt}")
                    nc.tensor.matmul(psY[:], la[0:65, tsl], wall_sb[:])

                    vals = smp.tile([128, 8], f32, tag="vals", name=f"v{b}_{rt}")
                    nc.vector.max(vals[:], gr[:])
                    # max_index of the previous row tile is emitted here, one
                    # tile late: its (coarsened) cross-engine wait is then
                    # already satisfied, keeping the DVE queue head unblocked.
                    if pend is not None:
                        pvals, pgr, prt = pend
                        nc.vector.max_index(
                            gall[:, prt * 8 : (prt + 1) * 8], pvals[:], pgr[:]
                        )
                    pend = (vals, gr, rt)

                    if rt % 4 == 0:
                        ybig = yp.tile(
                            [128, 4 * K * OUT_C], f16, tag="y16", name=f"y{b}_{rt}"
                        )
                    nc.scalar.copy(
                        y16s := ybig[:, (rt % 4) * 512 : (rt % 4 + 1) * 512], psY[:]
                    )
                    if rt % 4 == 3:
                        nc.sync.dma_start(
                            y_d[b][:, (rt - 3) * 512 : (rt + 1) * 512], ybig[:]
                        )

                pvals, pgr, prt = pend
                nc.vector.max_index(
                    gall[:, prt * 8 : (prt + 1) * 8], pvals[:], pgr[:]
                )
                nc.sync.dma_start(g_d[b], gall[:])

    nc.compile()
    return nc


def _get_nc():
    if "nc" not in _CACHE:
        _CACHE["nc"] = build_nc()
    return _CACHE["nc"]


def host_inputs(x, w, b):
    """Per-core input maps from full inputs."""
    x = np.asarray(x, dtype=np.float32)
    w = np.asarray(w, dtype=np.float32)
    b = np.asarray(b, dtype=np.float32)

    h = x.astype(np.float16)                                   # [B, C, T]
    l = (x.astype(np.float64) - h.astype(np.float64)).astype(np.float16)
    norm = np.sum(x.astype(np.float64) * x.astype(np.float64), axis=1)  # [B, T]
    nhi = norm.astype(np.float16)
    nlo = (norm - nhi.astype(np.float64)).astype(np.float16)
    cent = (100.0 - norm).astype(np.float16)
    ones = np.ones((B, 1, T), np.float16)

    la = np.concatenate(
        [2 * h, ones, ones, cent[:, None, :]], axis=1
    )  # [B, 67, T]
    ra = np.concatenate(
        [h, -nhi[:, None, :], -nlo[:, None, :], ones], axis=1
    )  # [B, 67, T]
    lb = np.concatenate([2 * l, 2 * h], axis=1)  # [B, 128, T]
    rb = np.concatenate([h, l], axis=1)          # [B, 128, T]

    wall = np.empty((65, K * OUT_C), np.float16)
    wall[:C] = (w.transpose(1, 2, 0).reshape(C, K * OUT_C) / 2).astype(np.float16)
    wall[C] = np.tile((b / K).astype(np.float16), K)

    return [
        {
            "la": np.ascontiguousarray(la[i * BPC : (i + 1) * BPC]),
            "ra": np.ascontiguousarray(ra[i * BPC : (i + 1) * BPC]),
            "lb": np.ascontiguousarray(lb[i * BPC : (i + 1) * BPC]),
            "rb": np.ascontiguousarray(rb[i * BPC : (i + 1) * BPC]),
            "wall": wall,
        }
        for i in range(NCORES)
    ]


def kernel(x, w, b):
    from concourse.bass_utils import run_bass_kernel_spmd

    x = np.asarray(x, dtype=np.float32)
    nc = _get_nc()
    in_maps = host_inputs(x, w, b)
    res = run_bass_kernel_spmd(nc, in_maps, list(range(NCORES)))

    out = np.empty((B, OUT_C, T), np.float32)
    jj = np.arange(K)[None, :]
    goff = np.arange(G)[None, None, :]
    for i in range(NCORES):
        yv = res.results[i]["yout"]   # [BPC, 128, RT*512] f16
        gi = res.results[i]["gidx"]   # [BPC, 128, RT*8] u16
        for bb in range(BPC):
            gb = i * BPC + bb
            # token t = rt*128 + p  ->  groups at gi[bb][p, rt*8 + j]
            g = gi[bb].reshape(128, RT, 8).transpose(1, 0, 2).reshape(T, 8)
            g = np.minimum(g.astype(np.int64), NG - 1)
            cand = (g[:, :, None] * G + goff).reshape(T, 8 * G)   # [T, 128]
            cand = np.sort(cand, axis=1)
            # exact re-rank (fp32, same formula as reference)
            xb = x[gb]                                  # [C, T]
            nb = np.sum(xb * xb, axis=0)                # [T]
            dots = np.einsum("ct,ctk->tk", xb, xb[:, cand])
            dist = nb[:, None] + nb[cand] - 2 * dots
            order = np.argsort(dist, axis=1, kind="stable")[:, :K]
            sel = np.take_along_axis(cand, order, axis=1)         # [T, K]
            # gather-sum the device conv taps (token t = rt*128 + p is
            # stored at yv[bb][p, rt*512:(rt+1)*512])
            yb = (
                yv[bb].reshape(128, RT, K * OUT_C).transpose(1, 0, 2)
                .reshape(T, K, OUT_C).astype(np.float32)
            )
            gath = yb[sel, jj, :]                                 # [T, K, OUT_C]
            out[gb] = gath.sum(1).T
    return out.astype(np.float32)
